# revision 1
# baseline (speedup 1.0000x reference)
"""FPN ROIAlign pooler (nn_Pooler) on 8 trn2 cores.

Strategy: data-parallel over RoIs (boxes dealt round-robin by level-group for
balance). Host builds a channels-last pixel table [161508px, 256ch] fp32 and
per-core gather index/weight streams. Device: for each 128-bin chunk,
dma_gather 1024 rows (8 gathers x 128 bins; gather = one (y-row, x-window) of
one bilinear corner pair), multiply by per-bin weights (DVE broadcast), reduce
over the 8 x window taps -> [128 bins, 256 ch], DMA out. Host reassembles
[1024, 256, 7, 7].
"""
import numpy as np
from contextlib import ExitStack

from concourse import bacc, bass, mybir, tile, bass_utils

C = 256
N_CORES = 8
OUT = 7
LVL_HW = [(200, 304), (100, 152), (50, 76), (25, 38)]
SCALES = (0.25, 0.125, 0.0625, 0.03125)
SEG_SZ = [h * w for h, w in LVL_HW]           # px per (lvl, batch) segment
# segment order: (0,0),(0,1),(1,0),(1,1),(2,0),(2,1),(3,0),(3,1)
SEG_BASE = np.zeros((4, 2), np.int64)
_off = 0
for _l in range(4):
    for _b in range(2):
        SEG_BASE[_l, _b] = _off
        _off += SEG_SZ[_l]
TOTAL_PX = int(_off)                           # 161500
END_PAD_PX = 8
TABLE_PX = TOTAL_PX + END_PAD_PX

# gather groups: (base_px, step_px, win_px, n_rows)
GROUPS = [
    (0, 2, 3, 30400),          # lvl0 batch0 (2px-stride rows, 3px window)
    (60800, 2, 3, 30400),      # lvl0 batch1
    (121600, 1, 2, 30400),     # lvl1 both batches
    (152000, 1, 2, 9500),      # lvl2+lvl3 all
]
GRP_WIN = [3, 3, 2, 2]

NQ = 1024          # gather slots per chunk (8 taps x 128 bins)
IDXC = NQ // 16    # idx columns per chunk

_nc_cache = {}


def _build_nc(chunks):
    nc = bacc.Bacc("TRN2", target_bir_lowering=False, debug=False,
                   num_devices=N_CORES)
    nch = sum(chunks)
    wcols = sum(8 * GRP_WIN[g] * chunks[g] for g in range(4))
    table_d = nc.dram_tensor("table", [TABLE_PX * C], mybir.dt.float32,
                             kind="ExternalInput")
    idx_d = nc.dram_tensor("idxs", [128, IDXC * nch], mybir.dt.int16,
                           kind="ExternalInput")
    w_d = nc.dram_tensor("wts", [128, wcols], mybir.dt.float32,
                         kind="ExternalInput")
    out_d = nc.dram_tensor("out", [nch * 128, C], mybir.dt.float32,
                           kind="ExternalOutput")

    with tile.TileContext(nc) as tc, ExitStack() as ctx:
        sbi = ctx.enter_context(tc.tile_pool(name="sbi", bufs=3))
        sbd = ctx.enter_context(tc.tile_pool(name="sbd", bufs=2))
        sbp = ctx.enter_context(tc.tile_pool(name="sbp", bufs=2))
        sbo = ctx.enter_context(tc.tile_pool(name="sbo", bufs=2))

        ci = 0
        woff = 0
        for g in range(4):
            base_px, step_px, win_px, n_rows = GROUPS[g]
            es = win_px * C
            in_ap = bass.AP(tensor=table_d, offset=base_px * C,
                            ap=[[step_px * C, n_rows], [1, es]])
            for _ in range(chunks[g]):
                idx_t = sbi.tile([128, IDXC], mybir.dt.int16)
                nc.default_dma_engine.dma_start(
                    out=idx_t[:], in_=idx_d.ap()[:, ci * IDXC:(ci + 1) * IDXC])
                dst_t = sbd.tile([128, 8, es], mybir.dt.float32)
                nc.gpsimd.dma_gather(dst_t[:], in_ap, idx_t[:], NQ, NQ, es,
                                     elem_step=step_px * C)
                w_t = sbi.tile([128, 8, win_px], mybir.dt.float32)
                nc.default_dma_engine.dma_start(
                    out=w_t[:].rearrange("p a b -> p (a b)"),
                    in_=w_d.ap()[:, woff:woff + 8 * win_px])
                prod_t = sbp.tile([128, 8, win_px, C], mybir.dt.float32)
                nc.vector.tensor_tensor(
                    out=prod_t[:],
                    in0=dst_t[:].rearrange("p t (x c) -> p t x c", x=win_px, c=C),
                    in1=w_t[:].unsqueeze(3).broadcast_to([128, 8, win_px, C]),
                    op=mybir.AluOpType.mult)
                out_t = sbo.tile([128, C], mybir.dt.float32)
                nc.vector.tensor_reduce(
                    out=out_t[:], in_=prod_t[:].transpose([0, 3, 1, 2]),
                    axis=mybir.AxisListType.XY, op=mybir.AluOpType.add)
                nc.default_dma_engine.dma_start(
                    out=out_d.ap()[ci * 128:(ci + 1) * 128, :], in_=out_t[:])
                ci += 1
                woff += 8 * win_px
    nc.compile()
    return nc


def _host_prep(f0, f1, f2, f3, boxes, bidx):
    boxes32 = np.asarray(boxes, np.float32)
    b = np.asarray(bidx).astype(np.int64)
    N = boxes32.shape[0]

    # level routing in strict fp32 (matches jax reference arithmetic)
    x1, y1, x2, y2 = (boxes32[:, k] for k in range(4))
    area = (x2 - x1 + np.float32(1.0)) * (y2 - y1 + np.float32(1.0))
    s = np.sqrt(area)
    lv = np.floor(np.float32(4.0) + np.log2(s / np.float32(224.0)
                                            + np.float32(1e-6)))
    lvl = (np.clip(lv, 2.0, 5.0)).astype(np.int64) - 2

    # channels-last flat table
    segs = []
    for f in (f0, f1, f2, f3):
        fa = np.asarray(f, np.float32)
        for bb in range(2):
            segs.append(np.transpose(fa[bb], (1, 2, 0)).reshape(-1, C))
    segs.append(np.zeros((END_PAD_PX, C), np.float32))
    table_flat = np.ascontiguousarray(np.concatenate(segs, 0)).reshape(-1)

    scs = np.array(SCALES)[lvl]
    Wl = np.array([hw[1] for hw in LVL_HW])[lvl]
    Hl = np.array([hw[0] for hw in LVL_HW])[lvl]
    x1s = boxes32[:, 0].astype(np.float64) * scs
    y1s = boxes32[:, 1].astype(np.float64) * scs
    x2s = boxes32[:, 2].astype(np.float64) * scs
    y2s = boxes32[:, 3].astype(np.float64) * scs
    bin_w = np.maximum(x2s - x1s, 1.0) / OUT
    bin_h = np.maximum(y2s - y1s, 1.0) / OUT
    grid = (np.arange(OUT)[:, None] + np.array([0.25, 0.75])[None, :]).reshape(-1)
    xs = x1s[:, None] + bin_w[:, None] * grid[None, :]     # [N,14]
    ys = y1s[:, None] + bin_h[:, None] * grid[None, :]
    vx = (xs >= -1.0) & (xs <= Wl[:, None])
    vy = (ys >= -1.0) & (ys <= Hl[:, None])
    xc = np.clip(xs, 0.0, (Wl - 1)[:, None])
    yc = np.clip(ys, 0.0, (Hl - 1)[:, None])
    x0c = np.minimum(np.floor(xc).astype(np.int64), (Wl - 2)[:, None])
    y0c = np.minimum(np.floor(yc).astype(np.int64), (Hl - 2)[:, None])
    lx = xc - x0c
    ly = yc - y0c

    seg_base = SEG_BASE[lvl, b]
    group = np.where(lvl == 0, b, np.where(lvl == 1, 2, 3))
    GRP_BASE_PX = np.array([0, 60800, 121600, 152000])
    gbase = GRP_BASE_PX[group]

    # addr[n, sy, t, sx]
    yrow = y0c[:, :, None] + np.arange(2)[None, None, :]            # [N,14,2]
    addr = (seg_base[:, None, None, None]
            + yrow[:, :, :, None] * Wl[:, None, None, None]
            + x0c[:, None, None, :])                                 # [N,14,2,14]
    local = addr - gbase[:, None, None, None]
    is0 = (lvl == 0)[:, None, None, None]
    row = np.where(is0, local // 2, local)
    par = np.where(is0, local % 2, 0)

    yw = np.stack([1.0 - ly, ly], axis=2)                            # [N,14,2]
    xw = np.stack([1.0 - lx, lx], axis=2)                            # [N,14,2]
    valid = (vy[:, :, None] & vx[:, None, :])                        # [N,14,14]
    base_w = valid[:, :, None, :] * yw[:, :, :, None] * 0.25         # [N,14,2,14]
    WMAX = 3
    wfull = np.zeros((N, 14, 2, 14, WMAX))
    for k in range(2):
        np.put_along_axis(
            wfull, (par + k)[..., None],
            (base_w * xw[:, None, None, :, k])[..., None], axis=4)

    # -> bins: sy=(by,iy), sx=(bx,ix); t8 = iy*4 + t*2 + ix
    idx_bin = (row.reshape(N, 7, 2, 2, 7, 2)
               .transpose(0, 1, 4, 2, 3, 5).reshape(N, 49, 8))
    w_bin = (wfull.reshape(N, 7, 2, 2, 7, 2, WMAX)
             .transpose(0, 1, 4, 2, 3, 5, 6).reshape(N, 49, 8, WMAX))

    for g in range(4):
        m = group == g
        if m.any():
            assert idx_bin[m].min() >= 0 and idx_bin[m].max() < GROUPS[g][3]

    # deal boxes: sort by group, core i takes sorted[i::8]
    order = np.argsort(group, kind="stable")
    core_boxes = [order[i::N_CORES] for i in range(N_CORES)]
    Bg = np.array([[np.sum(group[cb] == g) for g in range(4)]
                   for cb in core_boxes])
    maxb = Bg.max(axis=0)
    chunks = tuple(int(-(-49 * mb // 128)) for mb in maxb)

    idx_all, w_all, omap = [], [], []
    for core in range(N_CORES):
        cb = core_boxes[core]
        icols, wcols, cmap = [], [], []
        row_base = 0
        for g in range(4):
            bl = cb[group[cb] == g]
            nb = len(bl)
            tot = chunks[g] * 128
            win = GRP_WIN[g]
            ib = np.zeros((tot, 8), np.int64)
            wv = np.zeros((tot, 8, win))
            if nb:
                ib[:49 * nb] = idx_bin[bl].reshape(-1, 8)
                wv[:49 * nb] = w_bin[bl][..., :win].reshape(-1, 8, win)
            for c in range(chunks[g]):
                ic = ib[c * 128:(c + 1) * 128].T.reshape(NQ)     # slot=t8*128+q
                assert ic.max() < 32768
                wr = np.tile(ic.reshape(IDXC, 16).T.astype(np.int16), (8, 1))
                icols.append(wr)
                wcols.append(wv[c * 128:(c + 1) * 128]
                             .reshape(128, 8 * win).astype(np.float32))
            cmap.append((bl, row_base))
            row_base += tot
        idx_all.append(np.concatenate(icols, axis=1))
        w_all.append(np.concatenate(wcols, axis=1))
        omap.append(cmap)
    return table_flat, idx_all, w_all, omap, chunks


LAST_RESULT = None


def kernel(f0, f1, f2, f3, boxes, box_batch_idx):
    global LAST_RESULT
    table_flat, idx_all, w_all, omap, chunks = _host_prep(
        f0, f1, f2, f3, boxes, box_batch_idx)
    if chunks not in _nc_cache:
        _nc_cache[chunks] = _build_nc(chunks)
    nc = _nc_cache[chunks]
    in_maps = [{"table": table_flat, "idxs": idx_all[i], "wts": w_all[i]}
               for i in range(N_CORES)]
    res = bass_utils.run_bass_kernel_spmd(nc, in_maps,
                                          core_ids=list(range(N_CORES)))
    LAST_RESULT = res

    outfull = np.zeros((1024, 49, C), np.float32)
    for core in range(N_CORES):
        r = np.asarray(res.results[core]["out"])
        for (bl, row_base) in omap[core]:
            nb = len(bl)
            if nb:
                outfull[bl] = r[row_base:row_base + 49 * nb].reshape(nb, 49, C)
    return np.ascontiguousarray(
        outfull.transpose(0, 2, 1).reshape(1024, C, OUT, OUT))



# revision 3
# speedup vs baseline: 2.3476x; 2.3476x over previous
"""FPN ROIAlign pooler (nn_Pooler) on 8 trn2 cores — TensorEngine version.

Strategy: data-parallel over RoIs. Host builds a channels-last fp16 pixel
table [161500px, 256ch] and, per box, the ~180 distinct pixel-PAIRS its
7x7x(2x2) bilinear sampling grid touches plus a sparse weight matrix
W[slot, parity, 49bins] (fp16). Device: batched dma_gather pulls pair rows
(1KB each) into SBUF tiles [128 pairs, 512ch]; TensorE accumulates
P[49bins, 256ch] += W[:,t,q,:].T @ F[:,t,q*256:(q+1)*256] over the box's
tiles in PSUM; scalar engine copies PSUM->SBUF; DMA out. The vector engine
is idle by design (it was the v1 bottleneck at 87% busy).

Boxes are dealt to cores in rounds of 8 (one box per core per round),
sorted per level-group by descending tile count, so every core executes an
identical static instruction stream (SPMD) with per-core data.
"""
import numpy as np
from contextlib import ExitStack

from concourse import bacc, bass, mybir, tile, bass_utils

C = 256
N_CORES = 8
OUT = 7
NBIN = OUT * OUT
LVL_HW = [(200, 304), (100, 152), (50, 76), (25, 38)]
SCALES = (0.25, 0.125, 0.0625, 0.03125)
SEG_SZ = [h * w for h, w in LVL_HW]           # px per (lvl, batch) segment
# segment order: (0,0),(0,1),(1,0),(1,1),(2,0),(2,1),(3,0),(3,1)
SEG_BASE = np.zeros((4, 2), np.int64)
_off = 0
for _l in range(4):
    for _b in range(2):
        SEG_BASE[_l, _b] = _off
        _off += SEG_SZ[_l]
TOTAL_PX = int(_off)                           # 161500
END_PAD_PX = 4
TABLE_PX = TOTAL_PX + END_PAD_PX

# pair-gather groups: (base_px, n_pairs). Pair = 2 adjacent x-pixels.
PGROUPS = [
    (0, 30400),        # lvl0 batch0
    (60800, 30400),    # lvl0 batch1
    (121600, 15200),   # lvl1 both batches
    (152000, 4750),    # lvl2+lvl3 all
]
PAIR_ELEM = 2 * C                 # fp16 elements per pair row (1KB)
TBATCH = 8                        # max 128-pair tiles per dma_gather call
                                  # (1024 idxs = SWDGE descriptor carveout)

_nc_cache = {}


def _build_nc(sig):
    """sig: tuple of batches; each batch = (group, (tb_round0, tb_round1, ...))."""
    nc = bacc.Bacc("TRN2", target_bir_lowering=False, debug=False,
                   num_devices=N_CORES)
    rounds_total = sum(len(tbs) for _, tbs in sig)
    tiles_total = sum(sum(tbs) for _, tbs in sig)
    idx_cols = tiles_total * 8
    w_cols = tiles_total * 2 * NBIN

    table_d = nc.dram_tensor("table", [TABLE_PX * C], mybir.dt.float16,
                             kind="ExternalInput")
    idx_d = nc.dram_tensor("idxs", [128, idx_cols], mybir.dt.int16,
                           kind="ExternalInput")
    w_d = nc.dram_tensor("wts", [128, w_cols], mybir.dt.float16,
                         kind="ExternalInput")
    out_d = nc.dram_tensor("out", [rounds_total * NBIN, C], mybir.dt.float32,
                           kind="ExternalOutput")

    with tile.TileContext(nc) as tc, ExitStack() as ctx:
        sbi = ctx.enter_context(tc.tile_pool(name="sbi", bufs=1))
        sbf = ctx.enter_context(tc.tile_pool(name="sbf", bufs=3))
        sbw = ctx.enter_context(tc.tile_pool(name="sbw", bufs=3))
        sbo = ctx.enter_context(tc.tile_pool(name="sbo", bufs=3))
        psm = ctx.enter_context(tc.tile_pool(name="psm", bufs=4, space="PSUM"))

        idx_t = sbi.tile([128, idx_cols], mybir.dt.int16)
        nc.default_dma_engine.dma_start(out=idx_t[:], in_=idx_d.ap()[:, :])

        ioff = 0   # idx column offset
        woff = 0   # w column offset
        r = 0      # global round index
        for g, tbs in sig:
            base_px, npairs = PGROUPS[g]
            in_ap = bass.AP(tensor=table_d, offset=base_px * C,
                            ap=[[PAIR_ELEM, npairs], [1, PAIR_ELEM]])
            s_t = sum(tbs)
            nidx = s_t * 128
            f_t = sbf.tile([128, s_t, PAIR_ELEM], mybir.dt.float16)
            nc.gpsimd.dma_gather(f_t[:], in_ap,
                                 idx_t[:, ioff:ioff + nidx // 16],
                                 nidx, nidx, PAIR_ELEM, elem_step=PAIR_ELEM)
            w_t = sbw.tile([128, s_t, 2, NBIN], mybir.dt.float16)
            nc.default_dma_engine.dma_start(
                out=w_t[:].rearrange("p a b c -> p (a b c)"),
                in_=w_d.ap()[:, woff:woff + s_t * 2 * NBIN])
            n_r = len(tbs)
            o_t = sbo.tile([NBIN, n_r, C], mybir.dt.float32)
            toff = 0
            for k, tb in enumerate(tbs):
                p_t = psm.tile([NBIN, C], mybir.dt.float32)
                for t in range(tb):
                    for q in (0, 1):
                        nc.tensor.matmul(
                            p_t[:],
                            lhsT=w_t[:, toff + t, q, :],
                            rhs=f_t[:, toff + t, q * C:(q + 1) * C],
                            start=(t == 0 and q == 0),
                            stop=(t == tb - 1 and q == 1))
                nc.scalar.copy(out=o_t[:, k, :], in_=p_t[:])
                toff += tb
            out_ap = bass.AP(tensor=out_d, offset=r * NBIN * C,
                             ap=[[C, NBIN], [NBIN * C, n_r], [1, C]])
            nc.default_dma_engine.dma_start(out=out_ap, in_=o_t[:])
            r += n_r
            ioff += nidx // 16
            woff += s_t * 2 * NBIN
    nc.compile()
    return nc


def _host_prep(f0, f1, f2, f3, boxes, bidx):
    boxes32 = np.asarray(boxes, np.float32)
    b = np.asarray(bidx).astype(np.int64)
    N = boxes32.shape[0]

    # level routing in strict fp32 (matches jax reference arithmetic)
    x1, y1, x2, y2 = (boxes32[:, k] for k in range(4))
    area = (x2 - x1 + np.float32(1.0)) * (y2 - y1 + np.float32(1.0))
    s = np.sqrt(area)
    lv = np.floor(np.float32(4.0) + np.log2(s / np.float32(224.0)
                                            + np.float32(1e-6)))
    lvl = (np.clip(lv, 2.0, 5.0)).astype(np.int64) - 2

    # channels-last flat fp16 table
    segs = []
    for f in (f0, f1, f2, f3):
        fa = np.asarray(f, np.float32)
        for bb in range(2):
            segs.append(np.transpose(fa[bb], (1, 2, 0)).reshape(-1, C)
                        .astype(np.float16))
    segs.append(np.zeros((END_PAD_PX, C), np.float16))
    table_flat = np.ascontiguousarray(np.concatenate(segs, 0)).reshape(-1)

    scs = np.array(SCALES)[lvl]
    Wl = np.array([hw[1] for hw in LVL_HW])[lvl]
    Hl = np.array([hw[0] for hw in LVL_HW])[lvl]
    x1s = boxes32[:, 0].astype(np.float64) * scs
    y1s = boxes32[:, 1].astype(np.float64) * scs
    x2s = boxes32[:, 2].astype(np.float64) * scs
    y2s = boxes32[:, 3].astype(np.float64) * scs
    bin_w = np.maximum(x2s - x1s, 1.0) / OUT
    bin_h = np.maximum(y2s - y1s, 1.0) / OUT
    grid = (np.arange(OUT)[:, None] + np.array([0.25, 0.75])[None, :]).reshape(-1)
    xs = x1s[:, None] + bin_w[:, None] * grid[None, :]     # [N,14]
    ys = y1s[:, None] + bin_h[:, None] * grid[None, :]
    vx = (xs >= -1.0) & (xs <= Wl[:, None])
    vy = (ys >= -1.0) & (ys <= Hl[:, None])
    xc = np.clip(xs, 0.0, (Wl - 1)[:, None])
    yc = np.clip(ys, 0.0, (Hl - 1)[:, None])
    x0c = np.minimum(np.floor(xc).astype(np.int64), (Wl - 2)[:, None])
    y0c = np.minimum(np.floor(yc).astype(np.int64), (Hl - 2)[:, None])
    lx = xc - x0c
    ly = yc - y0c

    seg_base = SEG_BASE[lvl, b]
    group = np.where(lvl == 0, b, np.where(lvl == 1, 2, 3))
    gbase = np.array([pg[0] for pg in PGROUPS])[group]

    yw = np.stack([1.0 - ly, ly], axis=2)                  # [N,14,2]
    xw = np.stack([1.0 - lx, lx], axis=2)                  # [N,14,2]
    # contribution grid [N, 14sy, 2t, 14sx, 2u]
    yrow = y0c[:, :, None] + np.arange(2)[None, None, :]   # [N,14,2]
    px_glob = (seg_base[:, None, None, None, None]
               + yrow[:, :, :, None, None] * Wl[:, None, None, None, None]
               + x0c[:, None, None, :, None]
               + np.arange(2)[None, None, None, None, :])
    rel = px_glob - gbase[:, None, None, None, None]
    pair = rel // 2
    par = rel % 2
    wgt = ((vy[:, :, None, None, None] & vx[:, None, None, :, None])
           * yw[:, :, :, None, None] * xw[:, None, None, :, :] * 0.25)
    sy_i = np.arange(14)
    binid = np.broadcast_to(
        ((sy_i // 2)[:, None, None, None] * 7 + (sy_i // 2)[None, None, :, None]),
        (14, 2, 14, 2)).ravel()

    # per-box dedup -> (group, pairs_u, Wbox[nslots,2,49], tb)
    recs = []
    for n in range(N):
        pu, inv = np.unique(pair[n].ravel(), return_inverse=True)
        ns = len(pu)
        wb = np.zeros((ns, 2, NBIN))
        np.add.at(wb, (inv, par[n].ravel(), binid), wgt[n].ravel())
        g = int(group[n])
        assert pu.min() >= 0 and pu.max() < PGROUPS[g][1]
        recs.append((g, pu.astype(np.int64), wb, (ns + 127) // 128, n))

    # deal: per group, sort by tb desc (then nslots desc), pad to multiple
    # of 8 with dummies; round k takes sorted[8k:8k+8], core i gets 8k+i.
    rounds = []      # list of (group, tb_round, [8 recs])
    for g in range(4):
        gr = [rc for rc in recs if rc[0] == g]
        gr.sort(key=lambda rc: (-rc[3], -len(rc[1])))
        while len(gr) % 8:
            gr.append((g, np.zeros(1, np.int64),
                       np.zeros((1, 2, NBIN)), 1, -1))
        for k in range(len(gr) // 8):
            eight = gr[8 * k:8 * k + 8]
            tbr = max(rc[3] for rc in eight)
            rounds.append((g, tbr, eight))

    # batches of consecutive same-group rounds, <= TBATCH tiles each
    batches = []     # (group, [round indices])
    for ri, (g, tbr, _) in enumerate(rounds):
        if (batches and batches[-1][0] == g
                and sum(rounds[j][1] for j in batches[-1][1]) + tbr <= TBATCH):
            batches[-1][1].append(ri)
        else:
            batches.append((g, [ri]))
    sig = tuple((g, tuple(rounds[j][1] for j in rjs)) for g, rjs in batches)

    # per-core streams
    idx_all, w_all = [], []
    omap = []        # per core: list over global rounds of box id (-1 dummy)
    for core in range(N_CORES):
        icols, wcols, cmap = [], [], []
        for g, tbr, eight in rounds:
            _, pu, wb, _, box_id = eight[core]
            ns = len(pu)
            nsl = tbr * 128
            ic = np.full(nsl, pu[0], np.int64)
            ic[:ns] = pu
            assert ic.max() < 32768
            icols.append(np.tile(ic.reshape(tbr * 8, 16).T.astype(np.int16),
                                 (8, 1)))
            wp = np.zeros((nsl, 2, NBIN), np.float16)
            wp[:ns] = wb.astype(np.float16)
            wcols.append(wp.reshape(tbr, 128, 2 * NBIN)
                         .transpose(1, 0, 2).reshape(128, tbr * 2 * NBIN))
            cmap.append(box_id)
        idx_all.append(np.ascontiguousarray(np.concatenate(icols, axis=1)))
        w_all.append(np.ascontiguousarray(np.concatenate(wcols, axis=1)))
        omap.append(cmap)
    return table_flat, idx_all, w_all, omap, sig


LAST_RESULT = None


def kernel(f0, f1, f2, f3, boxes, box_batch_idx):
    global LAST_RESULT
    table_flat, idx_all, w_all, omap, sig = _host_prep(
        f0, f1, f2, f3, boxes, box_batch_idx)
    if sig not in _nc_cache:
        _nc_cache[sig] = _build_nc(sig)
    nc = _nc_cache[sig]
    in_maps = [{"table": table_flat, "idxs": idx_all[i], "wts": w_all[i]}
               for i in range(N_CORES)]
    res = bass_utils.run_bass_kernel_spmd(nc, in_maps,
                                          core_ids=list(range(N_CORES)))
    LAST_RESULT = res

    outfull = np.zeros((1024, NBIN, C), np.float32)
    for core in range(N_CORES):
        r = np.asarray(res.results[core]["out"])
        for ri, box_id in enumerate(omap[core]):
            if box_id >= 0:
                outfull[box_id] = r[ri * NBIN:(ri + 1) * NBIN]
    return np.ascontiguousarray(
        outfull.transpose(0, 2, 1).reshape(1024, C, OUT, OUT))
